# revision 32
# baseline (speedup 1.0000x reference)
"""Trainium2 Bass kernel for the masked-MSE actor-critic criterion.

Problem: inputs sample_seq/sample_value/sample_reward, all [65536, 256].
  mask[i, j] = 1 iff no zero appears in sample_seq[i, :j]  (prefix property)
  loss       = sum((reward-value)^2 * mask) / sum(mask)
  returns (loss, mean(reward-value), mean(reward))

Strategy (pure data-parallel over 8 NeuronCores):
  - Host shards the batch dim 8 ways and TRANSPOSES each shard to [S=256, 8192]
    so the sequence dim lies along SBUF partitions (2 blocks of 128).
  - All inputs go to the device as bf16 (seq values 0..19 are exact in bf16;
    bf16 reward/value perturb the loss by ~3e-6 relative, gate is 2e-2).
  - Per DMA tile of RD=4096 batch columns x 2 seq blocks:
      g  = (seq == 0)              VectorE tensor_scalar (4x mode)
      C0 = Tri^T @ g0              TensorE per 512-col PSUM chunk
      C1 = Tri^T @ g1 + Ones128^T @ g0   (all-ones lhsT broadcasts block0's
                                   zero-count to every row -- no copies)
      mask = relu(1 - C)           ScalarE, PSUM -> SBUF bf16,
                                   accum_out -> per-chunk sum(mask)
      d  = r - v; dk = d * mask    VectorE tensor_tensor (2x mode)
      dm = dk * dk                 (= d^2 * mask since mask is 0/1)
      sum(r), sum(d), sum(dm)      TensorE ones-matmuls, PSUM-accumulated
                                   across the whole kernel
  - Outputs per core: "sums" [1,3] (r/d/dm) + "acc" [128, nmask] mask sums.
    Host combines the 8 cores' partials into the 3 outputs.
"""

import numpy as np

B, S = 65536, 256
N_CORES = 8
P = 128
COLS = B // N_CORES  # 8192 columns (batch rows) per core
RD = 2048            # columns per DMA tile
R = 512              # columns per PSUM chunk (one bank / one matmul)

_cache = {}


def build_nc(cols, rd=RD, r=R):
    from concourse import bacc, tile, mybir

    dt = mybir.dt
    assert cols % rd == 0 and rd % r == 0
    ndma = cols // rd
    nchunk = rd // r
    nmask = 2 * (cols // r)      # per-chunk mask sums (2 seq blocks)

    nc = bacc.Bacc("TRN2", target_bir_lowering=False, debug=False,
                   num_devices=N_CORES)

    seq_d = nc.declare_dram_parameter("seq", [S, cols], dt.bfloat16, isOutput=False)
    rew_d = nc.declare_dram_parameter("rew", [S, cols], dt.bfloat16, isOutput=False)
    val_d = nc.declare_dram_parameter("val", [S, cols], dt.bfloat16, isOutput=False)
    tri_d = nc.declare_dram_parameter("tri", [P, P], dt.bfloat16, isOutput=False)
    onesm_d = nc.declare_dram_parameter("onesm", [P, P], dt.bfloat16, isOutput=False)
    ones_d = nc.declare_dram_parameter("ones", [P, 1], dt.bfloat16, isOutput=False)
    sums_d = nc.declare_dram_parameter("sums", [1, 3], dt.float32, isOutput=True)
    acc_d = nc.declare_dram_parameter("acc", [P, nmask], dt.float32, isOutput=True)

    AT = mybir.ActivationFunctionType
    OP = mybir.AluOpType
    NSEG = 3  # psum stat segments: 0=r, 1=d, 2=dm
    total_per_seg = ndma * nchunk * 2

    with tile.TileContext(nc) as tc:
        with (
            tc.tile_pool(name="const", bufs=1) as constp,
            tc.tile_pool(name="io", bufs=3) as iop,
            tc.tile_pool(name="mid", bufs=2) as midp,
            tc.tile_pool(name="accp", bufs=1) as accp,
            tc.tile_pool(name="cpsum", bufs=2, space="PSUM") as cpsump,
            tc.tile_pool(name="spsum", bufs=1, space="PSUM") as spsump,
            tc.tile_pool(name="outp", bufs=1) as outp,
        ):
            tri_t = constp.tile([P, P], dt.bfloat16)
            nc.sync.dma_start(tri_t[:], tri_d[:])
            onesm_t = constp.tile([P, P], dt.bfloat16)
            nc.sync.dma_start(onesm_t[:], onesm_d[:])
            ones_t = constp.tile([P, 1], dt.bfloat16)
            nc.sync.dma_start(ones_t[:], ones_d[:])

            acc = accp.tile([P, nmask], dt.float32, name="acc")
            stats = spsump.tile([1, NSEG * r], dt.float32)
            counts = {}

            def pe_sum(iseg, rhs_ap):
                k = counts.get(iseg, 0)
                counts[iseg] = k + 1
                nc.tensor.matmul(stats[0:1, iseg * r:(iseg + 1) * r], ones_t[:],
                                 rhs_ap, start=(k == 0),
                                 stop=(k == total_per_seg - 1),
                                 skip_group_check=True)

            for td in range(ndma):
                c0 = td * rd
                rs, vs, gs, masks, ds, dks = [], [], [], [], [], []
                for b in range(2):
                    pl, ph = b * P, (b + 1) * P
                    sq = iop.tile([P, rd], dt.bfloat16, tag=f"seq{b}")
                    rr = iop.tile([P, rd], dt.bfloat16, tag=f"r{b}")
                    vv = iop.tile([P, rd], dt.bfloat16, tag=f"v{b}")
                    # split DMA issue across two sequencers (each issue is
                    # ~600ns of serial sequencer time)
                    eng = nc.sync if b == 0 else nc.gpsimd
                    eng.dma_start(sq[:], seq_d[pl:ph, c0:c0 + rd])
                    eng.dma_start(rr[:], rew_d[pl:ph, c0:c0 + rd])
                    eng.dma_start(vv[:], val_d[pl:ph, c0:c0 + rd])
                    rs.append(rr); vs.append(vv)

                    g = midp.tile([P, rd], dt.bfloat16, tag=f"g{b}")
                    nc.vector.tensor_scalar(g[:], sq[:], 0.0, None, OP.is_equal)
                    gs.append(g)
                    mk = midp.tile([P, rd], dt.bfloat16, tag=f"mask{b}",
                                   name=f"mask{b}")
                    masks.append(mk)

                # prefix zero-counts + masks, per 512-col PSUM chunk
                for ch in range(nchunk):
                    sl = slice(ch * r, (ch + 1) * r)
                    mcol = (td * nchunk + ch) * 2
                    c0p = cpsump.tile([P, r], dt.float32, tag="c0p")
                    nc.tensor.matmul(c0p[:], tri_t[:], gs[0][:, sl])
                    c1p = cpsump.tile([P, r], dt.float32, tag="c1p")
                    nc.tensor.matmul(c1p[:], tri_t[:], gs[1][:, sl],
                                     start=True, stop=False)
                    nc.tensor.matmul(c1p[:], onesm_t[:], gs[0][:, sl],
                                     start=False, stop=True)
                    nc.scalar.activation(masks[0][:, sl], c0p[:], AT.Relu,
                                         bias=1.0, scale=-1.0,
                                         accum_out=acc[:, mcol:mcol + 1])
                    nc.scalar.activation(masks[1][:, sl], c1p[:], AT.Relu,
                                         bias=1.0, scale=-1.0,
                                         accum_out=acc[:, mcol + 1:mcol + 2])

                for b in range(2):
                    d = midp.tile([P, rd], dt.bfloat16, tag=f"d{b}")
                    nc.vector.tensor_tensor(d[:], rs[b][:], vs[b][:], OP.subtract)
                    ds.append(d)
                    dk = midp.tile([P, rd], dt.bfloat16, tag=f"dk{b}")
                    nc.vector.tensor_tensor(dk[:], d[:], masks[b][:], OP.mult)
                    dks.append(dk)
                    dms = midp.tile([P, rd], dt.bfloat16, tag=f"dms{b}")
                    nc.vector.tensor_tensor(dms[:], dk[:], dk[:], OP.mult)

                    # stat matmuls, batched densely per block
                    for ch in range(nchunk):
                        sl = slice(ch * r, (ch + 1) * r)
                        pe_sum(0, rs[b][:, sl])
                        pe_sum(1, d[:, sl])
                        pe_sum(2, dms[:, sl])

            sums_s = outp.tile([1, 3], dt.float32)
            stats3 = stats[:].rearrange("p (s r) -> p s r", s=NSEG)
            nc.vector.reduce_sum(sums_s[:].rearrange("p (s o) -> p s o", o=1),
                                 stats3, axis=mybir.AxisListType.X)
            nc.sync.dma_start(sums_d[:], sums_s[:])
            nc.sync.dma_start(acc_d[:], acc[:])

    nc.compile()
    meta = {"nmask": nmask}
    return nc, meta


def make_consts():
    import ml_dtypes
    bf16 = ml_dtypes.bfloat16
    # tri[k, j] = 1 if k < j  (strictly-lower prefix: C[j] = # zeros before j)
    tri = np.triu(np.ones((P, P), dtype=np.float32), 1).astype(bf16)
    onesm = np.ones((P, P), dtype=bf16)
    ones = np.ones((P, 1), dtype=bf16)
    return tri, onesm, ones


def prep_shards(sample_seq, sample_value, sample_reward):
    """Host-side shard prep: batch-shard 8 ways, transpose to [S, cols]."""
    import ml_dtypes
    bf16 = ml_dtypes.bfloat16
    seq_bf = np.asarray(sample_seq).astype(bf16)          # values in [0, 20)
    rew_bf = np.asarray(sample_reward).astype(bf16)
    val_bf = np.asarray(sample_value).astype(bf16)

    tri, onesm, ones = make_consts()
    in_maps = []
    for c in range(N_CORES):
        lo, hi = c * COLS, (c + 1) * COLS
        in_maps.append({
            "seq": np.ascontiguousarray(seq_bf[lo:hi].T),
            "rew": np.ascontiguousarray(rew_bf[lo:hi].T),
            "val": np.ascontiguousarray(val_bf[lo:hi].T),
            "tri": tri,
            "onesm": onesm,
            "ones": ones,
        })
    return in_maps


def combine(parts, meta):
    """parts: per-core dicts with 'sums' [1,3] (r/d/dm) and 'acc' mask sums."""
    sum_r = sum_mask = sum_d = sum_dm = 0.0
    for p in parts:
        sums = np.asarray(p["sums"], dtype=np.float64)[0]
        sum_r += sums[0]
        sum_d += sums[1]
        sum_dm += sums[2]
        sum_mask += np.asarray(p["acc"], dtype=np.float64).sum()
    n = float(B) * float(S)
    return np.array([sum_dm / sum_mask, sum_d / n, sum_r / n], dtype=np.float32)


def run(sample_seq, sample_value, sample_reward, trace=False, build_kwargs=None,
        **kwargs):
    from concourse.bass_utils import run_bass_kernel_spmd

    key = tuple(sorted((build_kwargs or {}).items()))
    if key not in _cache:
        _cache[key] = build_nc(COLS, **(build_kwargs or {}))
    nc, meta = _cache[key]

    in_maps = prep_shards(sample_seq, sample_value, sample_reward)
    res = run_bass_kernel_spmd(nc, in_maps, core_ids=list(range(N_CORES)),
                               trace=trace, **kwargs)
    return combine(res.results, meta), res


def kernel(sample_seq, sample_value, sample_reward):
    out, _ = run(sample_seq, sample_value, sample_reward)
    return out


# revision 33
# speedup vs baseline: 1.0924x; 1.0924x over previous
"""Trainium2 Bass kernel for the masked-MSE actor-critic criterion.

Problem: inputs sample_seq/sample_value/sample_reward, all [65536, 256].
  mask[i, j] = 1 iff no zero appears in sample_seq[i, :j]  (prefix property)
  loss       = sum((reward-value)^2 * mask) / sum(mask)
  returns (loss, mean(reward-value), mean(reward))

Strategy (pure data-parallel over 8 NeuronCores):
  - Host shards the batch dim 8 ways and TRANSPOSES each shard to [S=256, 8192]
    so the sequence dim lies along SBUF partitions (2 blocks of 128).
  - All inputs go to the device as bf16 (seq values 0..19 are exact in bf16;
    bf16 reward/value perturb the loss by ~3e-6 relative, gate is 2e-2).
  - Per DMA tile of RD=4096 batch columns x 2 seq blocks:
      g  = (seq == 0)              VectorE tensor_scalar (4x mode)
      C0 = Tri^T @ g0              TensorE per 512-col PSUM chunk
      C1 = Tri^T @ g1 + Ones128^T @ g0   (all-ones lhsT broadcasts block0's
                                   zero-count to every row -- no copies)
      mask = relu(1 - C)           ScalarE, PSUM -> SBUF bf16,
                                   accum_out -> per-chunk sum(mask)
      d  = r - v; dk = d * mask    VectorE tensor_tensor (2x mode)
      dm = dk * dk                 (= d^2 * mask since mask is 0/1)
      sum(r), sum(d), sum(dm)      TensorE ones-matmuls, PSUM-accumulated
                                   across the whole kernel
  - Outputs per core: "sums" [1,3] (r/d/dm) + "acc" [128, nmask] mask sums.
    Host combines the 8 cores' partials into the 3 outputs.
"""

import numpy as np

B, S = 65536, 256
N_CORES = 8
P = 128
COLS = B // N_CORES  # 8192 columns (batch rows) per core
RD = 2048            # columns per DMA tile
R = 512              # columns per PSUM chunk (one bank / one matmul)

_cache = {}


def build_nc(cols, rd=RD, r=R):
    from concourse import bacc, tile, mybir

    dt = mybir.dt
    assert cols % rd == 0 and rd % r == 0
    ndma = cols // rd
    nchunk = rd // r
    nmask = 2 * (cols // r)      # per-chunk mask sums (2 seq blocks)

    nc = bacc.Bacc("TRN2", target_bir_lowering=False, debug=False,
                   num_devices=N_CORES)

    seq_d = nc.declare_dram_parameter("seq", [S, cols], dt.bfloat16, isOutput=False)
    rew_d = nc.declare_dram_parameter("rew", [S, cols], dt.bfloat16, isOutput=False)
    val_d = nc.declare_dram_parameter("val", [S, cols], dt.bfloat16, isOutput=False)
    tri_d = nc.declare_dram_parameter("tri", [P, P], dt.bfloat16, isOutput=False)
    onesm_d = nc.declare_dram_parameter("onesm", [P, P], dt.bfloat16, isOutput=False)
    ones_d = nc.declare_dram_parameter("ones", [P, 1], dt.bfloat16, isOutput=False)
    sums_d = nc.declare_dram_parameter("sums", [1, 3], dt.float32, isOutput=True)
    acc_d = nc.declare_dram_parameter("acc", [P, nmask], dt.float32, isOutput=True)

    AT = mybir.ActivationFunctionType
    OP = mybir.AluOpType
    NSEG = 3  # psum stat segments: 0=r, 1=d, 2=dm
    total_per_seg = ndma * nchunk * 2

    with tile.TileContext(nc) as tc:
        with (
            tc.tile_pool(name="const", bufs=1) as constp,
            tc.tile_pool(name="io", bufs=3) as iop,
            tc.tile_pool(name="mid", bufs=2) as midp,
            tc.tile_pool(name="accp", bufs=1) as accp,
            tc.tile_pool(name="cpsum", bufs=2, space="PSUM") as cpsump,
            tc.tile_pool(name="spsum", bufs=1, space="PSUM") as spsump,
            tc.tile_pool(name="outp", bufs=1) as outp,
        ):
            tri_t = constp.tile([P, P], dt.bfloat16)
            nc.sync.dma_start(tri_t[:], tri_d[:])
            onesm_t = constp.tile([P, P], dt.bfloat16)
            nc.sync.dma_start(onesm_t[:], onesm_d[:])
            ones_t = constp.tile([P, 1], dt.bfloat16)
            nc.sync.dma_start(ones_t[:], ones_d[:])

            acc = accp.tile([P, nmask], dt.float32, name="acc")
            stats = spsump.tile([1, NSEG * r], dt.float32)
            counts = {}

            def pe_sum(iseg, rhs_ap):
                k = counts.get(iseg, 0)
                counts[iseg] = k + 1
                nc.tensor.matmul(stats[0:1, iseg * r:(iseg + 1) * r], ones_t[:],
                                 rhs_ap, start=(k == 0),
                                 stop=(k == total_per_seg - 1),
                                 skip_group_check=True)

            for td in range(ndma):
                c0 = td * rd
                rs, vs, gs, masks, ds, dks = [], [], [], [], [], []
                for b in range(2):
                    pl, ph = b * P, (b + 1) * P
                    sq = iop.tile([P, rd], dt.bfloat16, tag=f"seq{b}")
                    rr = iop.tile([P, rd], dt.bfloat16, tag=f"r{b}")
                    vv = iop.tile([P, rd], dt.bfloat16, tag=f"v{b}")
                    nc.sync.dma_start(sq[:], seq_d[pl:ph, c0:c0 + rd])
                    nc.sync.dma_start(rr[:], rew_d[pl:ph, c0:c0 + rd])
                    nc.sync.dma_start(vv[:], val_d[pl:ph, c0:c0 + rd])
                    rs.append(rr); vs.append(vv)

                    g = midp.tile([P, rd], dt.bfloat16, tag=f"g{b}")
                    nc.vector.tensor_scalar(g[:], sq[:], 0.0, None, OP.is_equal)
                    gs.append(g)
                    mk = midp.tile([P, rd], dt.bfloat16, tag=f"mask{b}",
                                   name=f"mask{b}")
                    masks.append(mk)

                # prefix zero-counts + masks, per 512-col PSUM chunk
                for ch in range(nchunk):
                    sl = slice(ch * r, (ch + 1) * r)
                    mcol = (td * nchunk + ch) * 2
                    c0p = cpsump.tile([P, r], dt.float32, tag="c0p")
                    nc.tensor.matmul(c0p[:], tri_t[:], gs[0][:, sl])
                    c1p = cpsump.tile([P, r], dt.float32, tag="c1p")
                    nc.tensor.matmul(c1p[:], tri_t[:], gs[1][:, sl],
                                     start=True, stop=False)
                    nc.tensor.matmul(c1p[:], onesm_t[:], gs[0][:, sl],
                                     start=False, stop=True)
                    nc.scalar.activation(masks[0][:, sl], c0p[:], AT.Relu,
                                         bias=1.0, scale=-1.0,
                                         accum_out=acc[:, mcol:mcol + 1])
                    nc.scalar.activation(masks[1][:, sl], c1p[:], AT.Relu,
                                         bias=1.0, scale=-1.0,
                                         accum_out=acc[:, mcol + 1:mcol + 2])

                for b in range(2):
                    d = midp.tile([P, rd], dt.bfloat16, tag=f"d{b}")
                    nc.vector.tensor_tensor(d[:], rs[b][:], vs[b][:], OP.subtract)
                    ds.append(d)
                    dk = midp.tile([P, rd], dt.bfloat16, tag=f"dk{b}")
                    nc.vector.tensor_tensor(dk[:], d[:], masks[b][:], OP.mult)
                    dks.append(dk)
                    dms = midp.tile([P, rd], dt.bfloat16, tag=f"dms{b}")
                    nc.vector.tensor_tensor(dms[:], dk[:], dk[:], OP.mult)

                    # stat matmuls, batched densely per block
                    for ch in range(nchunk):
                        sl = slice(ch * r, (ch + 1) * r)
                        pe_sum(0, rs[b][:, sl])
                        pe_sum(1, d[:, sl])
                        pe_sum(2, dms[:, sl])

            sums_s = outp.tile([1, 3], dt.float32)
            stats3 = stats[:].rearrange("p (s r) -> p s r", s=NSEG)
            nc.vector.reduce_sum(sums_s[:].rearrange("p (s o) -> p s o", o=1),
                                 stats3, axis=mybir.AxisListType.X)
            nc.sync.dma_start(sums_d[:], sums_s[:])
            nc.sync.dma_start(acc_d[:], acc[:])

    nc.compile()
    meta = {"nmask": nmask}
    return nc, meta


def make_consts():
    import ml_dtypes
    bf16 = ml_dtypes.bfloat16
    # tri[k, j] = 1 if k < j  (strictly-lower prefix: C[j] = # zeros before j)
    tri = np.triu(np.ones((P, P), dtype=np.float32), 1).astype(bf16)
    onesm = np.ones((P, P), dtype=bf16)
    ones = np.ones((P, 1), dtype=bf16)
    return tri, onesm, ones


def prep_shards(sample_seq, sample_value, sample_reward):
    """Host-side shard prep: batch-shard 8 ways, transpose to [S, cols]."""
    import ml_dtypes
    bf16 = ml_dtypes.bfloat16
    seq_bf = np.asarray(sample_seq).astype(bf16)          # values in [0, 20)
    rew_bf = np.asarray(sample_reward).astype(bf16)
    val_bf = np.asarray(sample_value).astype(bf16)

    tri, onesm, ones = make_consts()
    in_maps = []
    for c in range(N_CORES):
        lo, hi = c * COLS, (c + 1) * COLS
        in_maps.append({
            "seq": np.ascontiguousarray(seq_bf[lo:hi].T),
            "rew": np.ascontiguousarray(rew_bf[lo:hi].T),
            "val": np.ascontiguousarray(val_bf[lo:hi].T),
            "tri": tri,
            "onesm": onesm,
            "ones": ones,
        })
    return in_maps


def combine(parts, meta):
    """parts: per-core dicts with 'sums' [1,3] (r/d/dm) and 'acc' mask sums."""
    sum_r = sum_mask = sum_d = sum_dm = 0.0
    for p in parts:
        sums = np.asarray(p["sums"], dtype=np.float64)[0]
        sum_r += sums[0]
        sum_d += sums[1]
        sum_dm += sums[2]
        sum_mask += np.asarray(p["acc"], dtype=np.float64).sum()
    n = float(B) * float(S)
    return np.array([sum_dm / sum_mask, sum_d / n, sum_r / n], dtype=np.float32)


def run(sample_seq, sample_value, sample_reward, trace=False, build_kwargs=None,
        **kwargs):
    from concourse.bass_utils import run_bass_kernel_spmd

    key = tuple(sorted((build_kwargs or {}).items()))
    if key not in _cache:
        _cache[key] = build_nc(COLS, **(build_kwargs or {}))
    nc, meta = _cache[key]

    in_maps = prep_shards(sample_seq, sample_value, sample_reward)
    res = run_bass_kernel_spmd(nc, in_maps, core_ids=list(range(N_CORES)),
                               trace=trace, **kwargs)
    return combine(res.results, meta), res


def kernel(sample_seq, sample_value, sample_reward):
    out, _ = run(sample_seq, sample_value, sample_reward)
    return out


# revision 34
# speedup vs baseline: 1.3030x; 1.1928x over previous
"""Trainium2 Bass kernel for the masked-MSE actor-critic criterion.

Problem: inputs sample_seq/sample_value/sample_reward, all [65536, 256].
  mask[i, j] = 1 iff no zero appears in sample_seq[i, :j]  (prefix property)
  loss       = sum((reward-value)^2 * mask) / sum(mask)
  returns (loss, mean(reward-value), mean(reward))

Strategy (pure data-parallel over 8 NeuronCores):
  - Host shards the batch dim 8 ways and TRANSPOSES each shard to [S=256, 8192]
    so the sequence dim lies along SBUF partitions (2 blocks of 128).
  - All inputs go to the device as bf16 (seq values 0..19 are exact in bf16;
    bf16 reward/value perturb the loss by ~3e-6 relative, gate is 2e-2).
  - Per DMA tile of RD=4096 batch columns x 2 seq blocks:
      g  = (seq == 0)              VectorE tensor_scalar (4x mode)
      C0 = Tri^T @ g0              TensorE per 512-col PSUM chunk
      C1 = Tri^T @ g1 + Ones128^T @ g0   (all-ones lhsT broadcasts block0's
                                   zero-count to every row -- no copies)
      mask = relu(1 - C)           ScalarE, PSUM -> SBUF bf16,
                                   accum_out -> per-chunk sum(mask)
      d  = r - v; dk = d * mask    VectorE tensor_tensor (2x mode)
      dm = dk * dk                 (= d^2 * mask since mask is 0/1)
      sum(r), sum(d), sum(dm)      TensorE ones-matmuls, PSUM-accumulated
                                   across the whole kernel
  - Outputs per core: "sums" [1,3] (r/d/dm) + "acc" [128, nmask] mask sums.
    Host combines the 8 cores' partials into the 3 outputs.
"""

import numpy as np

B, S = 65536, 256
N_CORES = 8
P = 128
COLS = B // N_CORES  # 8192 columns (batch rows) per core
RD = 2048            # columns per DMA tile
R = 512              # columns per PSUM chunk (one bank / one matmul)

_cache = {}


def build_nc(cols, rd=RD, r=R):
    from concourse import bacc, tile, mybir

    dt = mybir.dt
    assert cols % rd == 0 and rd % r == 0
    ndma = cols // rd
    nchunk = rd // r
    nmask = 2 * (cols // r)      # per-chunk mask sums (2 seq blocks)

    nc = bacc.Bacc("TRN2", target_bir_lowering=False, debug=False,
                   num_devices=N_CORES)

    seq_d = nc.declare_dram_parameter("seq", [S, cols], dt.bfloat16, isOutput=False)
    rew_d = nc.declare_dram_parameter("rew", [S, cols], dt.bfloat16, isOutput=False)
    val_d = nc.declare_dram_parameter("val", [S, cols], dt.bfloat16, isOutput=False)
    tri_d = nc.declare_dram_parameter("tri", [P, P], dt.bfloat16, isOutput=False)
    onesm_d = nc.declare_dram_parameter("onesm", [P, P], dt.bfloat16, isOutput=False)
    ones_d = nc.declare_dram_parameter("ones", [P, 1], dt.bfloat16, isOutput=False)
    sums_d = nc.declare_dram_parameter("sums", [1, 3], dt.float32, isOutput=True)
    acc_d = nc.declare_dram_parameter("acc", [P, nmask], dt.float32, isOutput=True)

    AT = mybir.ActivationFunctionType
    OP = mybir.AluOpType
    NSEG = 3  # psum stat segments: 0=r, 1=d, 2=dm
    total_per_seg = ndma * nchunk * 2

    with tile.TileContext(nc) as tc:
        with (
            tc.tile_pool(name="const", bufs=1) as constp,
            tc.tile_pool(name="io", bufs=2) as iop,
            tc.tile_pool(name="mid", bufs=2) as midp,
            tc.tile_pool(name="accp", bufs=1) as accp,
            tc.tile_pool(name="cpsum", bufs=2, space="PSUM") as cpsump,
            tc.tile_pool(name="spsum", bufs=1, space="PSUM") as spsump,
            tc.tile_pool(name="outp", bufs=1) as outp,
        ):
            tri_t = constp.tile([P, P], dt.bfloat16)
            nc.sync.dma_start(tri_t[:], tri_d[:])
            onesm_t = constp.tile([P, P], dt.bfloat16)
            nc.sync.dma_start(onesm_t[:], onesm_d[:])
            ones_t = constp.tile([P, 1], dt.bfloat16)
            nc.sync.dma_start(ones_t[:], ones_d[:])

            acc = accp.tile([P, nmask], dt.float32, name="acc")
            stats = spsump.tile([1, NSEG * r], dt.float32)
            counts = {}

            def pe_sum(iseg, rhs_ap):
                k = counts.get(iseg, 0)
                counts[iseg] = k + 1
                nc.tensor.matmul(stats[0:1, iseg * r:(iseg + 1) * r], ones_t[:],
                                 rhs_ap, start=(k == 0),
                                 stop=(k == total_per_seg - 1),
                                 skip_group_check=True)

            for td in range(ndma):
                c0 = td * rd
                rs, vs, gs, masks, ds, dks = [], [], [], [], [], []
                for b in range(2):
                    pl, ph = b * P, (b + 1) * P
                    sq = iop.tile([P, rd], dt.bfloat16, tag=f"seq{b}")
                    rr = iop.tile([P, rd], dt.bfloat16, tag=f"r{b}")
                    vv = iop.tile([P, rd], dt.bfloat16, tag=f"v{b}")
                    nc.sync.dma_start(sq[:], seq_d[pl:ph, c0:c0 + rd])
                    nc.sync.dma_start(rr[:], rew_d[pl:ph, c0:c0 + rd])
                    nc.sync.dma_start(vv[:], val_d[pl:ph, c0:c0 + rd])
                    rs.append(rr); vs.append(vv)

                    g = midp.tile([P, rd], dt.bfloat16, tag=f"g{b}")
                    nc.vector.tensor_scalar(g[:], sq[:], 0.0, None, OP.is_equal)
                    gs.append(g)
                    mk = midp.tile([P, rd], dt.bfloat16, tag=f"mask{b}",
                                   name=f"mask{b}")
                    masks.append(mk)

                # prefix zero-counts + masks, per 512-col PSUM chunk
                for ch in range(nchunk):
                    sl = slice(ch * r, (ch + 1) * r)
                    mcol = (td * nchunk + ch) * 2
                    c0p = cpsump.tile([P, r], dt.float32, tag="c0p")
                    nc.tensor.matmul(c0p[:], tri_t[:], gs[0][:, sl])
                    c1p = cpsump.tile([P, r], dt.float32, tag="c1p")
                    nc.tensor.matmul(c1p[:], tri_t[:], gs[1][:, sl],
                                     start=True, stop=False)
                    nc.tensor.matmul(c1p[:], onesm_t[:], gs[0][:, sl],
                                     start=False, stop=True)
                    nc.scalar.activation(masks[0][:, sl], c0p[:], AT.Relu,
                                         bias=1.0, scale=-1.0,
                                         accum_out=acc[:, mcol:mcol + 1])
                    nc.scalar.activation(masks[1][:, sl], c1p[:], AT.Relu,
                                         bias=1.0, scale=-1.0,
                                         accum_out=acc[:, mcol + 1:mcol + 2])

                for b in range(2):
                    d = midp.tile([P, rd], dt.bfloat16, tag=f"d{b}")
                    nc.vector.tensor_tensor(d[:], rs[b][:], vs[b][:], OP.subtract)
                    ds.append(d)
                    dk = midp.tile([P, rd], dt.bfloat16, tag=f"dk{b}")
                    nc.vector.tensor_tensor(dk[:], d[:], masks[b][:], OP.mult)
                    dks.append(dk)
                    dms = midp.tile([P, rd], dt.bfloat16, tag=f"dms{b}")
                    nc.vector.tensor_tensor(dms[:], dk[:], dk[:], OP.mult)

                    # stat matmuls, batched densely per block
                    for ch in range(nchunk):
                        sl = slice(ch * r, (ch + 1) * r)
                        pe_sum(0, rs[b][:, sl])
                        pe_sum(1, d[:, sl])
                        pe_sum(2, dms[:, sl])

            sums_s = outp.tile([1, 3], dt.float32)
            stats3 = stats[:].rearrange("p (s r) -> p s r", s=NSEG)
            nc.vector.reduce_sum(sums_s[:].rearrange("p (s o) -> p s o", o=1),
                                 stats3, axis=mybir.AxisListType.X)
            nc.sync.dma_start(sums_d[:], sums_s[:])
            nc.sync.dma_start(acc_d[:], acc[:])

    nc.compile()
    meta = {"nmask": nmask}
    return nc, meta


def make_consts():
    import ml_dtypes
    bf16 = ml_dtypes.bfloat16
    # tri[k, j] = 1 if k < j  (strictly-lower prefix: C[j] = # zeros before j)
    tri = np.triu(np.ones((P, P), dtype=np.float32), 1).astype(bf16)
    onesm = np.ones((P, P), dtype=bf16)
    ones = np.ones((P, 1), dtype=bf16)
    return tri, onesm, ones


def prep_shards(sample_seq, sample_value, sample_reward):
    """Host-side shard prep: batch-shard 8 ways, transpose to [S, cols]."""
    import ml_dtypes
    bf16 = ml_dtypes.bfloat16
    seq_bf = np.asarray(sample_seq).astype(bf16)          # values in [0, 20)
    rew_bf = np.asarray(sample_reward).astype(bf16)
    val_bf = np.asarray(sample_value).astype(bf16)

    tri, onesm, ones = make_consts()
    in_maps = []
    for c in range(N_CORES):
        lo, hi = c * COLS, (c + 1) * COLS
        in_maps.append({
            "seq": np.ascontiguousarray(seq_bf[lo:hi].T),
            "rew": np.ascontiguousarray(rew_bf[lo:hi].T),
            "val": np.ascontiguousarray(val_bf[lo:hi].T),
            "tri": tri,
            "onesm": onesm,
            "ones": ones,
        })
    return in_maps


def combine(parts, meta):
    """parts: per-core dicts with 'sums' [1,3] (r/d/dm) and 'acc' mask sums."""
    sum_r = sum_mask = sum_d = sum_dm = 0.0
    for p in parts:
        sums = np.asarray(p["sums"], dtype=np.float64)[0]
        sum_r += sums[0]
        sum_d += sums[1]
        sum_dm += sums[2]
        sum_mask += np.asarray(p["acc"], dtype=np.float64).sum()
    n = float(B) * float(S)
    return np.array([sum_dm / sum_mask, sum_d / n, sum_r / n], dtype=np.float32)


def run(sample_seq, sample_value, sample_reward, trace=False, build_kwargs=None,
        **kwargs):
    from concourse.bass_utils import run_bass_kernel_spmd

    key = tuple(sorted((build_kwargs or {}).items()))
    if key not in _cache:
        _cache[key] = build_nc(COLS, **(build_kwargs or {}))
    nc, meta = _cache[key]

    in_maps = prep_shards(sample_seq, sample_value, sample_reward)
    res = run_bass_kernel_spmd(nc, in_maps, core_ids=list(range(N_CORES)),
                               trace=trace, **kwargs)
    return combine(res.results, meta), res


def kernel(sample_seq, sample_value, sample_reward):
    out, _ = run(sample_seq, sample_value, sample_reward)
    return out
